# revision 12
# baseline (speedup 1.0000x reference)
"""Trainium2 Bass kernel for ColBERT negative-CE loss (8 NeuronCores).

Sharding: data-parallel over query batches (16 per core); doc_embeddings
replicated to every core. Each core computes, for its 16 query batches:
  - per-query-token maxes over doc tokens for all 128 doc batches (mc)
  - per-query-token maxes over its own negative doc batch (negmax)
The tiny O(B^2) tail (token sums, softplus, CE) runs on host in float64.

Reduction strategy per 8-doc chunk (16 matmuls -> 8 two-bank PSUM tiles):
  - 2 tiles: DVE reduce_max straight from PSUM (f32, 1x rate)
  - 6 tiles: ScalarE copies PSUM -> one fp16 SBUF "mega" tile; DVE then runs
    three tensor_tensor max folds at 2x 16-bit rate + one short 1x reduce.
This balances ScalarE and VectorE, the only engines that can read PSUM.

Self-contained: hardcodes shapes from the problem spec.
"""

import numpy as np

_B, _Nq, _Nd, _D = 128, 32, 256, 128
_M = 8          # cores
_BL = _B // _M  # query batches per core = 16
_T = 0.02

# per-chunk tile roles: (g, jj) with query group g in 0..3, doc-pair jj in 0..1
_DIRECT = [(0, 0), (2, 0)]
_EVAC = [(g, jj) for g in range(4) for jj in range(2) if (g, jj) not in _DIRECT]

_CACHE = {}


def _build_program():
    """Trace + compile the per-core Bass program (same program for all cores)."""
    from contextlib import ExitStack

    import concourse.bacc as bacc
    import concourse.tile as tile
    from concourse import mybir

    f32 = mybir.dt.float32
    fp16 = mybir.dt.bfloat16  # fp16 crashes at full scale on this runtime; bf16 is safe

    nc = bacc.Bacc("TRN2", target_bir_lowering=False, debug=False, num_devices=_M)

    # DRAM I/O (per core)
    qT = nc.dram_tensor("qT", [128, 512], fp16, kind="ExternalInput").ap()
    docc = nc.dram_tensor("docc", [16, 128, 2048], fp16, kind="ExternalInput").ap()
    negc = nc.dram_tensor("negc", [2, 128, 2048], fp16, kind="ExternalInput").ap()
    maxcol = nc.dram_tensor("maxcol", [128, 512], f32, kind="ExternalOutput").ap()
    negmax = nc.dram_tensor("negmax", [32, 16], f32, kind="ExternalOutput").ap()

    with tile.TileContext(nc) as tc, ExitStack() as ctx:
        singles = ctx.enter_context(tc.tile_pool(name="singles", bufs=1))
        dpool = ctx.enter_context(tc.tile_pool(name="dchunk", bufs=3))
        megapool = ctx.enter_context(tc.tile_pool(name="mega", bufs=4))
        foldpool = ctx.enter_context(tc.tile_pool(name="fold", bufs=2))
        pspool = ctx.enter_context(tc.tile_pool(name="ps", bufs=4, space="PSUM"))

        qt = singles.tile([128, 512], fp16)
        nc.sync.dma_start(out=qt, in_=qT)

        # doc chunk tiles + DMAs are emitted inside the loop; neg loads are
        # deferred so the first chunks' DMAs go out first.
        ng0 = singles.tile([128, 2048], fp16, tag="ng0")
        ng1 = singles.tile([128, 2048], fp16, tag="ng1")
        ng = [ng0, ng1]

        mc = singles.tile([128, 512], f32, tag="mc")
        nm = singles.tile([32, 16], f32, tag="nm")

        def do_chunk(ch):
            dt_ = dpool.tile([128, 2048], fp16)
            nc.gpsimd.dma_start(out=dt_, in_=docc[ch])
            mega = megapool.tile([128, 6144], fp16)
            dps = []
            for (g, jj) in _DIRECT + _EVAC:
                ps = pspool.tile([128, 1024], f32, tag="ps")
                for h in range(2):
                    j = jj * 2 + h
                    nc.tensor.matmul(
                        ps[:, h * 512 : (h + 1) * 512],
                        qt[:, g * 128 : (g + 1) * 128],
                        dt_[:, j * 512 : (j + 1) * 512],
                        start=True,
                        stop=True,
                    )
                if (g, jj) in _DIRECT:
                    dps.append(ps)
                else:
                    bi = _EVAC.index((g, jj))
                    nc.scalar.copy(
                        out=mega[:, bi * 1024 : (bi + 1) * 1024], in_=ps
                    )
            # direct reduces first on DVE (only need PE) — fills the wait for
            # the chunk's ScalarE evacuations
            for k, ps in enumerate(dps):
                nc.vector.reduce_max(
                    out=mc[:, ch * 32 + 4 * k : ch * 32 + 4 * k + 4],
                    in_=ps[:].rearrange("p (c s) -> p c s", s=256),
                    axis=mybir.AxisListType.X,
                )
            # fold the 24 evac'd c-blocks (fold1 split so it can start after
            # only part of the evacuations have landed)
            v = mega[:].rearrange("p (c s) -> p c s", s=256)  # [128, 24, 256]
            f1 = foldpool.tile([128, 24, 128], fp16, tag="f1")
            nc.vector.tensor_max(
                out=f1[:, 0:12, :], in0=v[:, 0:12, 0:128], in1=v[:, 0:12, 128:256]
            )
            nc.vector.tensor_max(
                out=f1[:, 12:24, :], in0=v[:, 12:24, 0:128], in1=v[:, 12:24, 128:256]
            )
            f2 = foldpool.tile([128, 24, 64], fp16, tag="f2")
            nc.vector.tensor_max(out=f2, in0=f1[:, :, 0:64], in1=f1[:, :, 64:128])
            f3 = foldpool.tile([128, 24, 32], fp16, tag="f3")
            nc.vector.tensor_max(out=f3, in0=f2[:, :, 0:32], in1=f2[:, :, 32:64])
            f4 = foldpool.tile([128, 24, 16], fp16, tag="f4")
            nc.vector.tensor_max(out=f4, in0=f3[:, :, 0:16], in1=f3[:, :, 16:32])
            nc.vector.reduce_max(
                out=mc[:, ch * 32 + 8 : ch * 32 + 32],
                in_=f4,
                axis=mybir.AxisListType.X,
            )
            # stream this chunk's 32 result columns out
            nc.sync.dma_start(
                out=maxcol[:, ch * 32 : (ch + 1) * 32],
                in_=mc[:, ch * 32 : (ch + 1) * 32],
            )

        def do_negs():
            # 16 neg batches, 4 per two-bank PSUM tile ([32, 256] slots)
            for p in range(4):
                nps = pspool.tile([128, 1024], f32, tag="ps")
                for q in range(4):
                    b = 4 * p + q
                    nc.tensor.matmul(
                        nps[0:32, 256 * q : 256 * q + 256],
                        qt[:, 32 * b : 32 * b + 32],
                        ng[b // 8][:, 256 * (b % 8) : 256 * (b % 8) + 256],
                        start=True,
                        stop=True,
                    )
                nc.vector.reduce_max(
                    out=nm[:, 4 * p : 4 * p + 4],
                    in_=nps[0:32].rearrange("p (c s) -> p c s", s=256),
                    axis=mybir.AxisListType.X,
                )
            nc.sync.dma_start(out=negmax, in_=nm)

        for ch in range(3):
            do_chunk(ch)
        # neg loads go out after the first chunks' doc DMAs
        nc.sync.dma_start(out=ng0, in_=negc[0])
        nc.sync.dma_start(out=ng1, in_=negc[1])
        for ch in range(3, 8):
            do_chunk(ch)
        do_negs()
        for ch in range(8, 16):
            do_chunk(ch)

    nc.compile()
    return nc


def _get_program():
    if "nc" not in _CACHE:
        _CACHE["nc"] = _build_program()
    return _CACHE["nc"]


def _colmap():
    """mc column -> scores column (g*128 + c)."""
    cmap = np.empty(512, dtype=np.int64)
    for ch in range(16):
        for k, (g, jj) in enumerate(_DIRECT):
            for i in range(4):
                cmap[ch * 32 + 4 * k + i] = g * 128 + ch * 8 + 4 * jj + i
        for bi, (g, jj) in enumerate(_EVAC):
            for i in range(4):
                cmap[ch * 32 + 8 + 4 * bi + i] = g * 128 + ch * 8 + 4 * jj + i
    return cmap


def prep_inputs(query_embeddings, doc_embeddings, neg_doc_embeddings):
    """Host-side sharding + layout prep -> per-core input maps."""
    import ml_dtypes

    bf = ml_dtypes.bfloat16
    q = np.asarray(query_embeddings, dtype=np.float32).astype(bf)
    d = np.asarray(doc_embeddings, dtype=np.float32).astype(bf)
    n = np.asarray(neg_doc_embeddings, dtype=np.float32).astype(bf)

    # docs: [B, Nd, D] -> chunks [16, D, 8, Nd] flattened to [16, 128, 2048]
    docc = np.ascontiguousarray(
        d.reshape(16, 8, _Nd, _D).transpose(0, 3, 1, 2)
    ).reshape(16, 128, 2048)

    in_maps = []
    for i in range(_M):
        qs = q[i * _BL : (i + 1) * _BL]  # [16, 32, 128]
        qT = np.ascontiguousarray(qs.transpose(2, 0, 1)).reshape(128, 512)
        ns = n[i * _BL : (i + 1) * _BL]  # [16, 256, 128]
        negc = np.ascontiguousarray(
            ns.reshape(2, 8, _Nd, _D).transpose(0, 3, 1, 2)
        ).reshape(2, 128, 2048)
        in_maps.append({"qT": qT, "docc": docc, "negc": negc})
    return in_maps


def postprocess(results):
    """Combine per-core outputs into the scalar loss (float64 host math)."""
    cmap = _colmap()
    scores = np.empty((_B, _B), dtype=np.float64)
    neg = np.empty((_B,), dtype=np.float64)
    for i in range(_M):
        mc_raw = np.asarray(results[i]["maxcol"], dtype=np.float64)  # [128, 512]
        nm = np.asarray(results[i]["negmax"], dtype=np.float64)  # [32, 16]
        mc = np.empty_like(mc_raw)
        mc[:, cmap] = mc_raw
        # mc[p, g*128+c], p = m*32+n, local batch = 4g+m
        s4 = mc.reshape(4, 32, 4, 128).sum(axis=1)  # [m, g, c]
        scores[i * _BL : (i + 1) * _BL] = s4.transpose(1, 0, 2).reshape(16, 128)
        neg[i * _BL : (i + 1) * _BL] = nm.sum(axis=0)

    t = _T
    pos = np.diag(scores)
    term1 = np.logaddexp(0.0, (neg - pos) / t).mean()
    lg = scores / t
    m_ = lg.max(axis=1)
    lse = m_ + np.log(np.exp(lg - m_[:, None]).sum(axis=1))
    ce = (lse - np.diag(lg)).mean()
    return np.float32((term1 + ce) / 2.0)


def run_device(in_maps, **kwargs):
    from concourse.bass_utils import run_bass_kernel_spmd

    nc = _get_program()
    return run_bass_kernel_spmd(nc, in_maps, list(range(_M)), **kwargs)


def kernel(query_embeddings, doc_embeddings, neg_doc_embeddings):
    in_maps = prep_inputs(query_embeddings, doc_embeddings, neg_doc_embeddings)
    res = run_device(in_maps)
    return postprocess(res.results)


# revision 13
# speedup vs baseline: 1.0108x; 1.0108x over previous
"""Trainium2 Bass kernel for ColBERT negative-CE loss (8 NeuronCores).

Sharding: data-parallel over query batches (16 per core); doc_embeddings
replicated to every core. Each core computes, for its 16 query batches:
  - per-query-token maxes over doc tokens for all 128 doc batches (mc)
  - per-query-token maxes over its own negative doc batch (negmax)
The tiny O(B^2) tail (token sums, softplus, CE) runs on host in float64.

Reduction strategy per 8-doc chunk (16 matmuls -> 8 two-bank PSUM tiles):
  - 2 tiles: DVE reduce_max straight from PSUM (f32, 1x rate)
  - 6 tiles: ScalarE copies PSUM -> one fp16 SBUF "mega" tile; DVE then runs
    three tensor_tensor max folds at 2x 16-bit rate + one short 1x reduce.
This balances ScalarE and VectorE, the only engines that can read PSUM.

Self-contained: hardcodes shapes from the problem spec.
"""

import numpy as np

_B, _Nq, _Nd, _D = 128, 32, 256, 128
_M = 8          # cores
_BL = _B // _M  # query batches per core = 16
_T = 0.02

# per-chunk tile roles: (g, jj) with query group g in 0..3, doc-pair jj in 0..1
_DIRECT = [(0, 0), (2, 0)]
_EVAC = [(g, jj) for g in range(4) for jj in range(2) if (g, jj) not in _DIRECT]

_CACHE = {}


def _build_program():
    """Trace + compile the per-core Bass program (same program for all cores)."""
    from contextlib import ExitStack

    import concourse.bacc as bacc
    import concourse.tile as tile
    from concourse import mybir

    f32 = mybir.dt.float32
    fp16 = mybir.dt.bfloat16  # fp16 crashes at full scale on this runtime; bf16 is safe

    nc = bacc.Bacc("TRN2", target_bir_lowering=False, debug=False, num_devices=_M)

    # DRAM I/O (per core)
    qT = nc.dram_tensor("qT", [128, 512], fp16, kind="ExternalInput").ap()
    docc = nc.dram_tensor("docc", [16, 128, 2048], fp16, kind="ExternalInput").ap()
    negc = nc.dram_tensor("negc", [2, 128, 2048], fp16, kind="ExternalInput").ap()
    maxcol = nc.dram_tensor("maxcol", [128, 512], f32, kind="ExternalOutput").ap()
    negmax = nc.dram_tensor("negmax", [32, 16], f32, kind="ExternalOutput").ap()

    with tile.TileContext(nc) as tc, ExitStack() as ctx:
        singles = ctx.enter_context(tc.tile_pool(name="singles", bufs=1))
        dpool = ctx.enter_context(tc.tile_pool(name="dchunk", bufs=3))
        megapool = ctx.enter_context(tc.tile_pool(name="mega", bufs=3))
        foldpool = ctx.enter_context(tc.tile_pool(name="fold", bufs=2))
        pspool = ctx.enter_context(tc.tile_pool(name="ps", bufs=4, space="PSUM"))

        qt = singles.tile([128, 512], fp16)
        nc.sync.dma_start(out=qt, in_=qT)

        # doc chunk tiles + DMAs are emitted inside the loop; neg loads are
        # deferred so the first chunks' DMAs go out first.
        ng0 = singles.tile([128, 2048], fp16, tag="ng0")
        ng1 = singles.tile([128, 2048], fp16, tag="ng1")
        ng = [ng0, ng1]

        mc = singles.tile([128, 512], f32, tag="mc")
        nm = singles.tile([32, 16], f32, tag="nm")

        def do_chunk(ch):
            dt_ = dpool.tile([128, 2048], fp16)
            if ch == 0:
                # halves align with the jj matmul slices so the first
                # matmuls start before the whole chunk has landed
                nc.sync.dma_start(out=dt_[:, 0:1024], in_=docc[ch][:, 0:1024])
                nc.sync.dma_start(out=dt_[:, 1024:2048], in_=docc[ch][:, 1024:2048])
            else:
                nc.sync.dma_start(out=dt_, in_=docc[ch])
            mega = megapool.tile([128, 6144], fp16)
            dps = []
            for (g, jj) in _DIRECT + _EVAC:
                ps = pspool.tile([128, 1024], f32, tag="ps")
                for h in range(2):
                    j = jj * 2 + h
                    nc.tensor.matmul(
                        ps[:, h * 512 : (h + 1) * 512],
                        qt[:, g * 128 : (g + 1) * 128],
                        dt_[:, j * 512 : (j + 1) * 512],
                        start=True,
                        stop=True,
                    )
                if (g, jj) in _DIRECT:
                    dps.append(ps)
                else:
                    bi = _EVAC.index((g, jj))
                    nc.scalar.copy(
                        out=mega[:, bi * 1024 : (bi + 1) * 1024], in_=ps
                    )
            # direct reduces first on DVE (only need PE) — fills the wait for
            # the chunk's ScalarE evacuations
            for k, ps in enumerate(dps):
                nc.vector.reduce_max(
                    out=mc[:, ch * 32 + 4 * k : ch * 32 + 4 * k + 4],
                    in_=ps[:].rearrange("p (c s) -> p c s", s=256),
                    axis=mybir.AxisListType.X,
                )
            # fold the 24 evac'd c-blocks (fold1 split so it can start after
            # only part of the evacuations have landed)
            v = mega[:].rearrange("p (c s) -> p c s", s=256)  # [128, 24, 256]
            f1 = foldpool.tile([128, 24, 128], fp16, tag="f1")
            nc.vector.tensor_max(
                out=f1[:, 0:12, :], in0=v[:, 0:12, 0:128], in1=v[:, 0:12, 128:256]
            )
            nc.vector.tensor_max(
                out=f1[:, 12:24, :], in0=v[:, 12:24, 0:128], in1=v[:, 12:24, 128:256]
            )
            f2 = foldpool.tile([128, 24, 64], fp16, tag="f2")
            nc.vector.tensor_max(out=f2, in0=f1[:, :, 0:64], in1=f1[:, :, 64:128])
            f3 = foldpool.tile([128, 24, 32], fp16, tag="f3")
            nc.vector.tensor_max(out=f3, in0=f2[:, :, 0:32], in1=f2[:, :, 32:64])
            f4 = foldpool.tile([128, 24, 16], fp16, tag="f4")
            nc.vector.tensor_max(out=f4, in0=f3[:, :, 0:16], in1=f3[:, :, 16:32])
            nc.vector.reduce_max(
                out=mc[:, ch * 32 + 8 : ch * 32 + 32],
                in_=f4,
                axis=mybir.AxisListType.X,
            )
            # stream this chunk's 32 result columns out
            nc.sync.dma_start(
                out=maxcol[:, ch * 32 : (ch + 1) * 32],
                in_=mc[:, ch * 32 : (ch + 1) * 32],
            )

        def do_negs():
            # 16 neg batches, 4 per two-bank PSUM tile ([32, 256] slots)
            for p in range(4):
                nps = pspool.tile([128, 1024], f32, tag="ps")
                for q in range(4):
                    b = 4 * p + q
                    nc.tensor.matmul(
                        nps[0:32, 256 * q : 256 * q + 256],
                        qt[:, 32 * b : 32 * b + 32],
                        ng[b // 8][:, 256 * (b % 8) : 256 * (b % 8) + 256],
                        start=True,
                        stop=True,
                    )
                nc.vector.reduce_max(
                    out=nm[:, 4 * p : 4 * p + 4],
                    in_=nps[0:32].rearrange("p (c s) -> p c s", s=256),
                    axis=mybir.AxisListType.X,
                )
            nc.sync.dma_start(out=negmax, in_=nm)

        for ch in range(3):
            do_chunk(ch)
        # neg loads go out after the first chunks' doc DMAs
        nc.sync.dma_start(out=ng0, in_=negc[0])
        nc.sync.dma_start(out=ng1, in_=negc[1])
        for ch in range(3, 8):
            do_chunk(ch)
        do_negs()
        for ch in range(8, 16):
            do_chunk(ch)

    nc.compile()
    return nc


def _get_program():
    if "nc" not in _CACHE:
        _CACHE["nc"] = _build_program()
    return _CACHE["nc"]


def _colmap():
    """mc column -> scores column (g*128 + c)."""
    cmap = np.empty(512, dtype=np.int64)
    for ch in range(16):
        for k, (g, jj) in enumerate(_DIRECT):
            for i in range(4):
                cmap[ch * 32 + 4 * k + i] = g * 128 + ch * 8 + 4 * jj + i
        for bi, (g, jj) in enumerate(_EVAC):
            for i in range(4):
                cmap[ch * 32 + 8 + 4 * bi + i] = g * 128 + ch * 8 + 4 * jj + i
    return cmap


def prep_inputs(query_embeddings, doc_embeddings, neg_doc_embeddings):
    """Host-side sharding + layout prep -> per-core input maps."""
    import ml_dtypes

    bf = ml_dtypes.bfloat16
    q = np.asarray(query_embeddings, dtype=np.float32).astype(bf)
    d = np.asarray(doc_embeddings, dtype=np.float32).astype(bf)
    n = np.asarray(neg_doc_embeddings, dtype=np.float32).astype(bf)

    # docs: [B, Nd, D] -> chunks [16, D, 8, Nd] flattened to [16, 128, 2048]
    docc = np.ascontiguousarray(
        d.reshape(16, 8, _Nd, _D).transpose(0, 3, 1, 2)
    ).reshape(16, 128, 2048)

    in_maps = []
    for i in range(_M):
        qs = q[i * _BL : (i + 1) * _BL]  # [16, 32, 128]
        qT = np.ascontiguousarray(qs.transpose(2, 0, 1)).reshape(128, 512)
        ns = n[i * _BL : (i + 1) * _BL]  # [16, 256, 128]
        negc = np.ascontiguousarray(
            ns.reshape(2, 8, _Nd, _D).transpose(0, 3, 1, 2)
        ).reshape(2, 128, 2048)
        in_maps.append({"qT": qT, "docc": docc, "negc": negc})
    return in_maps


def postprocess(results):
    """Combine per-core outputs into the scalar loss (float64 host math)."""
    cmap = _colmap()
    scores = np.empty((_B, _B), dtype=np.float64)
    neg = np.empty((_B,), dtype=np.float64)
    for i in range(_M):
        mc_raw = np.asarray(results[i]["maxcol"], dtype=np.float64)  # [128, 512]
        nm = np.asarray(results[i]["negmax"], dtype=np.float64)  # [32, 16]
        mc = np.empty_like(mc_raw)
        mc[:, cmap] = mc_raw
        # mc[p, g*128+c], p = m*32+n, local batch = 4g+m
        s4 = mc.reshape(4, 32, 4, 128).sum(axis=1)  # [m, g, c]
        scores[i * _BL : (i + 1) * _BL] = s4.transpose(1, 0, 2).reshape(16, 128)
        neg[i * _BL : (i + 1) * _BL] = nm.sum(axis=0)

    t = _T
    pos = np.diag(scores)
    term1 = np.logaddexp(0.0, (neg - pos) / t).mean()
    lg = scores / t
    m_ = lg.max(axis=1)
    lse = m_ + np.log(np.exp(lg - m_[:, None]).sum(axis=1))
    ce = (lse - np.diag(lg)).mean()
    return np.float32((term1 + ce) / 2.0)


def run_device(in_maps, **kwargs):
    from concourse.bass_utils import run_bass_kernel_spmd

    nc = _get_program()
    return run_bass_kernel_spmd(nc, in_maps, list(range(_M)), **kwargs)


def kernel(query_embeddings, doc_embeddings, neg_doc_embeddings):
    in_maps = prep_inputs(query_embeddings, doc_embeddings, neg_doc_embeddings)
    res = run_device(in_maps)
    return postprocess(res.results)
